# revision 1
# baseline (speedup 1.0000x reference)
"""FAGCN message-passing kernel for 8 Trainium2 NeuronCores.

Strategy (edge-parallel via dst-ownership):
  - Nodes are assigned to the 8 cores snake-wise in degree-sorted order, so
    every core owns ~N/8 nodes, ~E/8 edges, and sees the same degree profile
    (the compiled SPMD program is shared; only the index inputs differ).
  - Gate decomposition: tanh(Linear([h_dst, h_src])) = tanh(p1[dst] + p2[src] + b)
    with p1 = x @ w_dst, p2 = x @ w_src.  Phase 1 computes per-node scalars
    (p2, norm, p1+b) on device (PE transpose + matmul) into a small table.
  - Phase 2: per 128-node tile (dst-major, degree-sorted so slot padding is
    tiny), indirect-DMA gathers pull x[src] rows plus (p2, norm)[src] pairs
    and the tile's own (norm, p1b) pairs.  The gate is ACT tanh with a
    per-partition bias; aggregation is a DVE multiply + strided reduce.
    z[dst] = norm[dst] * sum_s tanh(p1b[dst] + p2[src]) * norm[src] * x[src].
"""

import os
import sys

sys.path.insert(0, "/opt/trn_rl_repo")

import numpy as np

P = 128

# Set to "bf16" to gather x rows from a bf16 copy of x (halves gather traffic,
# ~2e-3 relative error).  "f32" is exact.
GATHER_DTYPE = os.environ.get("FAGCN_GATHER_DTYPE", "f32")

LAST_RESULTS = None  # BassKernelResults of the most recent HW run (for profiling)


def _ceil_to(a, m):
    return ((a + m - 1) // m) * m


class Plan:
    pass


def _prep(x, gate_w, gate_b, src, dst, ncores=8):
    """Host-side sharding: shapes/constants + per-core input maps."""
    x = np.asarray(x, dtype=np.float32)
    gate_w = np.asarray(gate_w, dtype=np.float32)
    gate_b = np.asarray(gate_b, dtype=np.float32)
    src = np.asarray(src).astype(np.int64)
    dst = np.asarray(dst).astype(np.int64)

    N, D = x.shape
    assert D == 64
    E = src.shape[0]

    pl = Plan()
    pl.N, pl.D, pl.E, pl.ncores = N, D, E, ncores
    # phase-1 processes node rows in chunks of 128, paired for PE transposes;
    # at least one pad row is kept so it can serve as the zero-gate sentinel
    pl.NPAD = _ceil_to(N + 1, 2 * P)
    pl.CH = pl.NPAD // P
    # scal row of node n under the partition-major layout r(n) = (n%128)*CH + n//128;
    # node NPAD-1 maps to row NPAD-1.  Its deg is set to 1e30 so norm ~ 0.
    pl.SENT = pl.NPAD - 1

    deg = np.bincount(dst, minlength=N).astype(np.int64)

    # snake assignment over degree-sorted nodes -> per-core node lists
    order = np.argsort(-deg, kind="stable")
    n8 = _ceil_to(N, ncores)
    order_p = np.concatenate([order, np.full(n8 - N, -1, dtype=np.int64)])
    blocks = order_p.reshape(-1, ncores).copy()
    blocks[1::2] = blocks[1::2, ::-1]
    core_nodes = np.ascontiguousarray(blocks.T)  # [ncores, npc]
    npc = core_nodes.shape[1]
    pl.NPC_PAD = _ceil_to(npc, P)
    pl.TILES = pl.NPC_PAD // P
    pad = np.full((ncores, pl.NPC_PAD - npc), -1, dtype=np.int64)
    core_nodes = np.concatenate([core_nodes, pad], axis=1)  # [ncores, NPC_PAD]
    pl.core_nodes = core_nodes

    node_deg = np.where(core_nodes >= 0, deg[np.clip(core_nodes, 0, N - 1)], 0)
    deg_tiles = node_deg.reshape(ncores, pl.TILES, P)
    Kt = deg_tiles.max(axis=(0, 2)).astype(np.int64)
    Kt = np.maximum(Kt, 1)
    pl.Kt = Kt
    pl.SX = int(Kt.sum())

    # CSR by dst
    e_order = np.argsort(dst, kind="stable")
    src_sorted = src[e_order]
    ends = np.cumsum(deg)
    starts = ends - deg

    CH = pl.CH

    def r_of(n):  # scalar-table row for node n (partition-major layout)
        return (n % P) * CH + n // P

    # shared inputs
    xp = np.zeros((pl.NPAD, D), dtype=np.float32)
    xp[:N] = x
    wrep = np.empty((P, 128), dtype=np.float32)
    wrep[:, 0:64] = gate_w[0, 64:128][None, :]   # w_src
    wrep[:, 64:128] = gate_w[0, 0:64][None, :]   # w_dst
    b128 = np.full((P, 1), float(np.asarray(gate_b).reshape(-1)[0]), dtype=np.float32)
    degp = np.full(pl.NPAD, 1e30, dtype=np.float32)  # pad rows -> norm ~ 0
    degp[:N] = deg
    degt = np.ascontiguousarray(degp.reshape(CH, P).T)

    in_maps = []
    karange = np.arange(int(Kt.max()))[None, :]
    for c in range(ncores):
        # one slot-column stream per tile: [own | slot1..slotK], values are
        # xaug-table rows r(node) (partition-major layout)
        idx = np.full((P, pl.TILES + pl.SX), pl.SENT, dtype=np.int32)
        koff = 0
        for t in range(pl.TILES):
            K = int(Kt[t])
            nodes = core_nodes[c, t * P : (t + 1) * P]  # [128]
            real = nodes >= 0
            d = np.where(real, deg[np.clip(nodes, 0, N - 1)], 0)
            st = np.where(real, starts[np.clip(nodes, 0, N - 1)], 0)
            mask = karange[:, :K] < d[:, None]  # [128, K]
            pos = st[:, None] + karange[:, :K]
            vals = src_sorted[np.minimum(pos, E - 1)]
            idx[:, koff] = np.where(
                real, r_of(np.clip(nodes, 0, N - 1)), pl.SENT
            ).astype(np.int32)
            idx[:, koff + 1 : koff + 1 + K] = np.where(
                mask, r_of(vals), pl.SENT
            ).astype(np.int32)
            koff += 1 + K
        in_maps.append(
            {
                "xp": xp,
                "wrep": wrep,
                "b128": b128,
                "degt": degt,
                "idx": idx,
            }
        )
    return pl, in_maps


def _build_nc(pl):
    """Build the shared SPMD Bass/Tile program."""
    import concourse.bass as bass
    import concourse.bacc as bacc
    import concourse.mybir as mybir
    import concourse.tile as tile
    from concourse.masks import make_identity

    f32 = mybir.dt.float32
    i32 = mybir.dt.int32
    AF = mybir.ActivationFunctionType
    OP = mybir.AluOpType

    D = pl.D
    CH = pl.CH
    TILES = pl.TILES
    Kt = [int(k) for k in pl.Kt]
    SX = pl.SX

    # Bacc (not raw Bass): its compile() runs move_matmul_waits_to_ldweights +
    # nop/event-semaphore legalization — without it walrus rejects PE
    # instructions carrying >1 sync wait ("Too many sync wait commands").
    nc = bacc.Bacc("TRN2", target_bir_lowering=False, debug=False, num_devices=pl.ncores)
    xp_d = nc.dram_tensor("xp", [pl.NPAD, D], f32, kind="ExternalInput")
    wrep_d = nc.dram_tensor("wrep", [P, 128], f32, kind="ExternalInput")
    b128_d = nc.dram_tensor("b128", [P, 1], f32, kind="ExternalInput")
    degt_d = nc.dram_tensor("degt", [P, CH], f32, kind="ExternalInput")
    idx_d = nc.dram_tensor("idx", [P, SX + TILES], i32, kind="ExternalInput")
    z_d = nc.dram_tensor("z", [pl.NPC_PAD, D], f32, kind="ExternalOutput")
    # combined per-node row table, partition-major row order r(n)=(n%128)*CH+n//128:
    # [x (64) | p2 | norm | p1+b | pad]
    FA = 68
    xaug_d = nc.dram_tensor("xaug", [pl.NPAD, FA], f32)
    # gathers read a consolidated copy so they wait on ONE DMA completion
    # (multi-sem waits on the qPoolDynamic indirect instructions crash the HW)
    xaug2_d = nc.dram_tensor("xaug2", [pl.NPAD, FA], f32)

    # batched phase-2 gathers: group tiles while sum(K) <= BATCH_K
    BATCH_K = 128
    batches = []
    b0 = 0
    while b0 < TILES:
        b1 = b0 + 1
        ks = Kt[b0]
        while b1 < TILES and ks + Kt[b1] <= BATCH_K:
            ks += Kt[b1]
            b1 += 1
        batches.append((b0, b1, ks))
        b0 = b1

    with tile.TileContext(nc) as tc:
        with (
            tc.tile_pool(name="consts", bufs=1) as cpool,
            tc.tile_pool(name="ph1", bufs=3) as p1pool,
            tc.tile_pool(name="ph1ps", bufs=4, space="PSUM") as ps_t,
            tc.tile_pool(name="ph1pp", bufs=2, space="PSUM") as ps_p,
            tc.tile_pool(name="gather", bufs=2) as gpool,
            tc.tile_pool(name="work", bufs=3) as wpool,
        ):
            wrep_sb = cpool.tile([P, 128], f32)
            nc.sync.dma_start(out=wrep_sb[:], in_=wrep_d[:, :])
            b128_sb = cpool.tile([P, 1], f32)
            nc.sync.dma_start(out=b128_sb[:], in_=b128_d[:, :])

            # ---- norms for all nodes: norm = sqrt(1 / max(deg, 1)) ----
            degt_sb = cpool.tile([P, CH], f32)
            nc.sync.dma_start(out=degt_sb[:], in_=degt_d[:, :])
            dclip = cpool.tile([P, CH], f32)
            nc.vector.tensor_scalar(
                out=dclip[:], in0=degt_sb[:], scalar1=1.0, scalar2=None, op0=OP.max
            )
            rec = cpool.tile([P, CH], f32)
            nc.vector.reciprocal(out=rec[:], in_=dclip[:])
            normT = cpool.tile([P, CH], f32)
            nc.scalar.activation(out=normT[:], in_=rec[:], func=AF.Sqrt)

            # xaug rows viewed as [partition, chunk, FA]
            xaug_v = xaug_d[0 : pl.NPAD, :].rearrange("(p c) f -> p c f", p=P)

            # ---- phase 1: per-node [x | p2 | norm | p1+b] table (DVE dots) ----
            BC = 8  # chunks per batch
            for c0 in range(0, CH, BC):
                cn = min(BC, CH - c0)
                xa = p1pool.tile([P, BC * FA], f32, tag="xa")
                xav = xa[:].rearrange("p (i f) -> p i f", f=FA)
                nc.sync.dma_start(
                    out=xav[:, 0:cn, 0:64],
                    in_=xp_d[c0 * P : (c0 + cn) * P, :].rearrange(
                        "(t p) f -> p t f", t=cn
                    ),
                )
                tmp = p1pool.tile([P, BC * 64], f32, tag="tmp")
                tv = tmp[:].rearrange("p (i f) -> p i f", f=64)
                # p2 = x . w_src   (wrep rows 0:  w_src broadcast per partition)
                nc.vector.tensor_tensor(
                    out=tv[:, 0:cn, :],
                    in0=xav[:, 0:cn, 0:64],
                    in1=wrep_sb[:, 0:64]
                    .rearrange("p (o f) -> p o f", o=1)
                    .to_broadcast([P, cn, 64]),
                    op=OP.mult,
                )
                nc.vector.tensor_reduce(
                    out=xav[:, 0:cn, 64],
                    in_=tv[:, 0:cn, :],
                    axis=mybir.AxisListType.X,
                    op=OP.add,
                )
                # p1 = x . w_dst
                nc.vector.tensor_tensor(
                    out=tv[:, 0:cn, :],
                    in0=xav[:, 0:cn, 0:64],
                    in1=wrep_sb[:, 64:128]
                    .rearrange("p (o f) -> p o f", o=1)
                    .to_broadcast([P, cn, 64]),
                    op=OP.mult,
                )
                red1 = wpool.tile([P, BC], f32, tag="red1")
                nc.vector.tensor_reduce(
                    out=red1[:, 0:cn],
                    in_=tv[:, 0:cn, :],
                    axis=mybir.AxisListType.X,
                    op=OP.add,
                )
                nc.vector.tensor_scalar(
                    out=xav[:, 0:cn, 66],
                    in0=red1[:, 0:cn],
                    scalar1=b128_sb[:, 0:1],
                    scalar2=None,
                    op0=OP.add,
                )
                nc.vector.tensor_copy(out=xav[:, 0:cn, 65], in_=normT[:, c0 : c0 + cn])
                nc.vector.memset(xav[:, 0:cn, 67], 0.0)
                nc.sync.dma_start(out=xaug_v[:, c0 : c0 + cn, :], in_=xa[:, 0 : cn * FA])

            nc.sync.dma_start(out=xaug2_d[:, :], in_=xaug_d[:, :])
            if bool(int(os.environ.get("FAGCN_SKIP_P2", "0"))):
                batches = []

            # ---- phase 2: gather + gate + aggregate ----
            idx_sb = cpool.tile([P, SX + TILES], i32)
            nc.sync.dma_start(out=idx_sb[:], in_=idx_d[:, :])

            coff = [0]
            for k in Kt:
                coff.append(coff[-1] + 1 + k)

            for b0, b1, ks in batches:
                nt = b1 - b0
                s0 = coff[b0]
                ncols = ks + nt  # own slots included
                ga = gpool.tile([P, (BATCH_K + 8) * FA], f32, tag="ga")
                # one [P,1] indirect per slot column (only validated HW shape)
                for col in range(ncols):
                    nc.gpsimd.indirect_dma_start(
                        out=ga[:, col * FA : (col + 1) * FA],
                        out_offset=None,
                        in_=xaug2_d[:, :],
                        in_offset=bass.IndirectOffsetOnAxis(
                            ap=idx_sb[:, s0 + col : s0 + col + 1], axis=0
                        ),
                    )
                koff = 0
                for t in range(b0, b1):
                    K = Kt[t]
                    own = ga[:, koff * FA : koff * FA + FA]
                    gsl = ga[:, (koff + 1) * FA : (koff + 1 + K) * FA].rearrange(
                        "p (k f) -> p k f", f=FA
                    )
                    tin = wpool.tile([P, K], f32, tag="tin")
                    nc.vector.tensor_scalar(
                        out=tin[:],
                        in0=gsl[:, :, 64],
                        scalar1=own[:, 66:67],
                        scalar2=None,
                        op0=OP.add,
                    )
                    tt = wpool.tile([P, K], f32, tag="tt")
                    nc.scalar.activation(out=tt[:], in_=tin[:], func=AF.Tanh)
                    ee = wpool.tile([P, K], f32, tag="ee")
                    nc.vector.tensor_tensor(
                        out=ee[:], in0=tt[:], in1=gsl[:, :, 65], op=OP.mult
                    )
                    m = wpool.tile([P, K * 64], f32, tag="m")
                    eev = (
                        ee[:]
                        .rearrange("p (k o) -> p k o", o=1)
                        .to_broadcast([P, K, 64])
                    )
                    nc.vector.tensor_tensor(
                        out=m[:], in0=gsl[:, :, 0:64], in1=eev, op=OP.mult
                    )
                    red = wpool.tile([P, 64], f32, tag="red")
                    nc.vector.tensor_reduce(
                        out=red[:],
                        in_=m[:].rearrange("p (k f) -> p f k", f=64),
                        axis=mybir.AxisListType.X,
                        op=OP.add,
                    )
                    zt = wpool.tile([P, 64], f32, tag="zt")
                    nc.vector.tensor_scalar(
                        out=zt[:],
                        in0=red[:],
                        scalar1=own[:, 65:66],
                        scalar2=None,
                        op0=OP.mult,
                    )
                    nc.sync.dma_start(out=z_d[t * P : (t + 1) * P, :], in_=zt[:])
                    koff += 1 + K
    nc.compile()
    return nc


_BUILD_CACHE = {}


def build(x, gate_w, gate_b, src, dst, ncores=8):
    pl, in_maps = _prep(x, gate_w, gate_b, src, dst, ncores)
    key = (pl.N, pl.E, pl.ncores, GATHER_DTYPE, tuple(int(k) for k in pl.Kt))
    nc = _BUILD_CACHE.get(key)
    if nc is None:
        nc = _build_nc(pl)
        _BUILD_CACHE[key] = nc
    return pl, in_maps, nc


def _assemble(pl, outs):
    N, D = pl.N, pl.D
    z = np.zeros((N, D), dtype=np.float32)
    npc_real = pl.core_nodes.shape[1]
    for c in range(pl.ncores):
        nodes = pl.core_nodes[c]
        real = nodes >= 0
        z[nodes[real]] = outs[c][real]
    return z


def kernel(x, gate_w, gate_b, src, dst):
    global LAST_RESULTS
    from concourse.bass_utils import run_bass_kernel_spmd

    pl, in_maps, nc = build(x, gate_w, gate_b, src, dst)
    res = run_bass_kernel_spmd(
        nc,
        in_maps,
        core_ids=list(range(pl.ncores)),
        trace=bool(int(os.environ.get("FAGCN_TRACE", "0"))),
    )
    LAST_RESULTS = res
    outs = [r["z"] for r in res.results]
    return _assemble(pl, outs)



# revision 2
# speedup vs baseline: 1.0363x; 1.0363x over previous
"""FAGCN message-passing kernel for 8 Trainium2 NeuronCores — v2.

Measured-design notes (see exp/):
  - SWDGE dma_gather costs ~8.6ns/row of Q7 desc-gen; rotating 4 SWDGE
    queues gives ~2.8x. 1024 idxs/instruction is the validated max.
  - int16 gather indices span <32768 rows -> fp16 table [x(64)|p2] at 256B
    stride, 4 color ranges of 25088 rows. A host greedy colors nodes to
    balance per-dst in-edge colors (slot padding ~1.42x).
  - dst-sharded cores (snake over degree-sorted); per-(tile,color) K is the
    max across all 8 cores so the SPMD program is identical everywhere.
  - Phase 2 runs per group of tiles: gathers (color-major, ragged per-tile
    K), one PE matmul replicates p1b[dst] per column, DVE does
    gate/messages, equal-K-run tree adds reduce each tile's columns.
"""

import os
import sys

sys.path.insert(0, "/opt/trn_rl_repo")

import numpy as np

P = 128
NCORES = 8
D = 64
ELEM = 65            # fp16 payload per table row: x(64) + p2
FS = 128             # table row stride in fp16 elems (256B)
NCOLOR = 4
CPP = 32
IPC = 784
RSZ = CPP * IPC      # 25088 rows per color range
NPAD = NCOLOR * RSZ  # 100352
NUMG = 1024
CMAX = 320

LAST_RESULTS = None


def _ceil_to(a, m):
    return ((a + m - 1) // m) * m


class Plan:
    pass


# ---------------------------------------------------------------------------
# host prep (structure-only)
# ---------------------------------------------------------------------------

def _color_nodes(src, dst, N, rng):
    """Color nodes 0..3 balancing per-dst in-edge color counts.
    Chunked greedy sweeps (sum objective), then overflow-penalized sweeps."""
    cap = RSZ - 1
    deg = np.bincount(dst, minlength=N)
    tgt = np.ceil(np.concatenate([deg, [0]]) / NCOLOR).astype(np.int32)
    outdeg = np.bincount(src, minlength=N)
    mo = int(outdeg.max())
    # padded out-adjacency
    oo = np.argsort(src, kind="stable")
    dsort = dst[oo]
    ost = np.zeros(N + 1, np.int64)
    np.cumsum(np.bincount(src[oo], minlength=N), out=ost[1:])
    adj = np.zeros((N, mo), np.int64)
    msk = np.arange(mo)[None, :] < outdeg[:, None]
    adj[msk] = dsort
    adj[~msk] = N  # sentinel row in padded cnt

    color = rng.integers(0, NCOLOR, size=N).astype(np.int8)
    cnt = np.zeros((N + 1, NCOLOR), np.int32)
    np.add.at(cnt, (dst, color[src]), 1)
    CH = 512
    ar = np.arange(CH)
    for sweep in range(7):
        BIG = 0 if sweep < 3 else 1024
        perm = rng.permutation(N)
        for i0 in range(0, N, CH):
            nodes = perm[i0 : i0 + CH]
            nn = len(nodes)
            ds = adj[nodes]                      # [nn, mo]
            vm = msk[nodes]                      # [nn, mo]
            cn = cnt[ds]                         # [nn, mo, 4]
            cur = color[nodes]
            # remove own contribution from current color
            base = cn.copy()
            jj, ll = np.nonzero(vm)
            base[jj, ll, cur[jj]] -= 1
            if BIG:
                t = tgt[ds][:, :, None]
                sc = ((np.maximum(base + 1 - t, 0) * BIG + base + 1)
                      * vm[:, :, None]).sum(axis=1)
            else:
                sc = (base * vm[:, :, None]).sum(axis=1)
            best = np.argmin(sc, axis=1).astype(np.int8)
            better = sc[ar[:nn], best] < sc[ar[:nn], cur]
            chg = np.nonzero(better & (best != cur))[0]
            if len(chg) == 0:
                continue
            chn = nodes[chg]
            dsc = adj[chn][msk[chn]]
            np.subtract.at(cnt, (dsc, np.repeat(cur[chg], outdeg[chn])), 1)
            np.add.at(cnt, (dsc, np.repeat(best[chg], outdeg[chn])), 1)
            color[chn] = best[chg]
    # capacity repair
    sizes = np.bincount(color, minlength=NCOLOR)
    for c in range(NCOLOR):
        while sizes[c] > cap:
            idx = np.where(color == c)[0]
            tgtc = int(np.argmin(sizes))
            over = idx[: sizes[c] - cap]
            color[over] = tgtc
            sizes = np.bincount(color, minlength=NCOLOR)
    return color


def _prep(x, gate_w, gate_b, src, dst):
    x = np.asarray(x, dtype=np.float32)
    gate_w = np.asarray(gate_w, dtype=np.float32)
    gate_b = np.asarray(gate_b, dtype=np.float32)
    src = np.asarray(src).astype(np.int64)
    dst = np.asarray(dst).astype(np.int64)
    N = x.shape[0]
    E = src.shape[0]
    rng = np.random.default_rng(12345)

    deg = np.bincount(dst, minlength=N).astype(np.int64)
    norm = np.clip(deg, 1.0, None) ** -0.5

    color = _color_nodes(src, dst, N, rng)
    row_of = np.full(N, -1, dtype=np.int64)
    for c in range(NCOLOR):
        nodes_c = np.where(color == c)[0]
        row_of[nodes_c] = c * RSZ + 1 + np.arange(len(nodes_c))
    assert (row_of >= 0).all() and (row_of % RSZ > 0).all()

    xp = np.zeros((NPAD, D), dtype=np.float32)
    xp[row_of] = x

    order = np.argsort(-deg, kind="stable")
    n8 = _ceil_to(N, NCORES)
    order_p = np.concatenate([order, np.full(n8 - N, -1, dtype=np.int64)])
    blocks = order_p.reshape(-1, NCORES).copy()
    blocks[1::2] = blocks[1::2, ::-1]
    core_nodes = np.ascontiguousarray(blocks.T)
    npc = core_nodes.shape[1]
    NPC_PAD = _ceil_to(npc, P)
    TILES = NPC_PAD // P
    pad = np.full((NCORES, NPC_PAD - npc), -1, dtype=np.int64)
    core_nodes = np.concatenate([core_nodes, pad], axis=1)

    cnt = np.zeros((N, NCOLOR), dtype=np.int32)
    np.add.at(cnt, (dst, color[src]), 1)
    for c in range(NCORES):
        nodes = core_nodes[c]
        real = nodes >= 0
        cc = cnt[np.clip(nodes, 0, N - 1)] * real[:, None]
        key = np.lexsort((cc[:, 3], cc[:, 2], cc[:, 1], cc[:, 0], ~real))
        core_nodes[c] = nodes[key]

    cnt_cores = np.zeros((NCORES, NPC_PAD, NCOLOR), dtype=np.int32)
    for c in range(NCORES):
        nodes = core_nodes[c]
        real = nodes >= 0
        cnt_cores[c] = cnt[np.clip(nodes, 0, N - 1)] * real[:, None]
    Ktc = np.zeros((TILES, NCOLOR), dtype=np.int64)
    for t in range(TILES):
        Ktc[t] = np.maximum(
            cnt_cores[:, t * P : (t + 1) * P, :].max(axis=(0, 1)), 1
        )

    # groups of tiles, capped by total columns
    groups = []
    t0 = 0
    while t0 < TILES:
        t1 = t0 + 1
        cols = int(Ktc[t0].sum())
        while t1 < TILES and cols + int(Ktc[t1].sum()) <= CMAX:
            cols += int(Ktc[t1].sum())
            t1 += 1
        groups.append((t0, t1))
        t0 = t1
    ngrp = len(groups)

    pl = Plan()
    pl.N, pl.E = N, E
    pl.NPC_PAD, pl.TILES = NPC_PAD, TILES
    pl.core_nodes = core_nodes
    pl.groups = groups
    pl.Ktc = Ktc

    # column layout per group: for color c: for tile t: Ktc[t,c] columns
    # gathers: per (group, color) chunked at NUMG idxs, queue balanced
    grp_cols = []
    col_of_grp = [0]
    colmap = []  # per group: list over colors of list of (t, colbase_in_grp)
    gathers = []  # (grp, color, colbase_in_grp, ncols, qcol_off16, queue)
    qload = [0, 0, 0, 0]
    qcol = [0, 0, 0, 0]
    for g, (a, b) in enumerate(groups):
        cols = 0
        cm = []
        for cidx in range(NCOLOR):
            tl = []
            for t in range(a, b):
                tl.append((t, cols))
                cols += int(Ktc[t, cidx])
            cm.append(tl)
        colmap.append(cm)
        grp_cols.append(cols)
        col_of_grp.append(col_of_grp[-1] + cols)
        # gather chunks per color
        for cidx in range(NCOLOR):
            cstart = cm[cidx][0][1]
            cend = cm[cidx][-1][1] + int(Ktc[b - 1, cidx])
            g0 = cstart
            while g0 < cend:
                g1 = min(g0 + NUMG // P, cend)
                qn = int(np.argmin(qload))
                qload[qn] += g1 - g0
                gathers.append((g, cidx, g0, g1 - g0, qcol[qn], qn))
                qcol[qn] += (g1 - g0) * P // 16
                g0 = g1
    pl.grp_cols = grp_cols
    pl.col_of_grp = col_of_grp
    pl.colmap = colmap
    pl.gathers = gathers
    pl.CTOT = col_of_grp[-1]
    pl.IDX16 = max(qcol)

    # per-core streams
    ekey = dst * NCOLOR + color[src]
    e_order = np.argsort(ekey, kind="stable")
    src_sorted = src[e_order]
    cum = np.zeros(N * NCOLOR + 1, dtype=np.int64)
    np.cumsum(np.bincount(ekey, minlength=N * NCOLOR), out=cum[1:])

    wrep16 = np.broadcast_to(
        gate_w[0, D : 2 * D].astype(np.float16), (P, D)
    ).copy()
    wrep32 = np.broadcast_to(gate_w[0, 0:D], (P, D)).copy()
    b128 = np.full((P, 1), float(gate_b.reshape(-1)[0]), dtype=np.float32)

    # column -> (tile, color, k) tables (shared across cores)
    col_tile = np.zeros(pl.CTOT, dtype=np.int64)
    col_cidx = np.zeros(pl.CTOT, dtype=np.int64)
    col_k = np.zeros(pl.CTOT, dtype=np.int64)
    for g, (a, b) in enumerate(groups):
        base = col_of_grp[g]
        for cidx in range(NCOLOR):
            for (t, cb) in colmap[g][cidx]:
                K = int(Ktc[t, cidx])
                col_tile[base + cb : base + cb + K] = t
                col_cidx[base + cb : base + cb + K] = cidx
                col_k[base + cb : base + cb + K] = np.arange(K)

    in_maps = []
    for c in range(NCORES):
        nodes = core_nodes[c]
        nodes_cl = np.clip(nodes, 0, N - 1)
        real = nodes >= 0
        # vectorized per-column source rows
        lane_nodes = nodes_cl.reshape(TILES, P)
        lane_real = real.reshape(TILES, P)
        tcol = col_tile                                 # [CTOT]
        base_e = cum[lane_nodes[tcol] * NCOLOR + col_cidx[:, None]]
        cnt_e = (
            cum[lane_nodes[tcol] * NCOLOR + col_cidx[:, None] + 1] - base_e
        )
        has = lane_real[tcol] & (col_k[:, None] < cnt_e)
        e_idx = base_e + np.minimum(
            col_k[:, None], np.maximum(cnt_e - 1, 0)
        )
        s_nodes = src_sorted[e_idx]                     # [CTOT, P]
        rows = row_of[s_nodes] - col_cidx[:, None] * RSZ
        ids_all = np.where(has, rows, 0).astype(np.int16)  # [CTOT, P]
        npr = np.where(
            has, norm[s_nodes] * norm[lane_nodes[tcol]], 0.0
        ).T.astype(np.float16).copy()                   # [P, CTOT]
        ind = np.zeros((P, pl.CTOT), dtype=np.float16)
        ind[col_tile, np.arange(pl.CTOT)] = 1.0

        idx16 = np.zeros((P, pl.IDX16), dtype=np.int16)
        for (g, cidx, cb, nc_, io, qn) in gathers:
            c0 = col_of_grp[g] + cb
            flat = ids_all[c0 : c0 + nc_].reshape(-1)
            n16 = len(flat) // 16
            wrapped = flat.reshape(n16, 16).T
            pb = qn * 32
            idx16[pb : pb + 16, io : io + n16] = wrapped
            idx16[pb + 16 : pb + 32, io : io + n16] = wrapped

        xown = np.zeros((NPC_PAD, D), dtype=np.float32)
        xown[real] = x[nodes_cl[real]]

        in_maps.append(
            {
                "xp": xp,
                "wrep16": wrep16,
                "wrep32": wrep32,
                "b128": b128,
                "xown": xown,
                "idx16": idx16,
                "npr": npr,
                "ind": ind,
            }
        )
    return pl, in_maps


# ---------------------------------------------------------------------------
# numpy emulation (prep validation)
# ---------------------------------------------------------------------------

def emulate(pl, in_maps):
    outs = []
    for c in range(NCORES):
        mm = in_maps[c]
        tabx = mm["xp"].astype(np.float16)
        p2 = (
            tabx.astype(np.float32) @ mm["wrep16"][0].astype(np.float32)
        ).astype(np.float16)
        p1b = (mm["xown"] @ mm["wrep32"][0] + mm["b128"][0, 0]).astype(
            np.float32
        )
        z = np.zeros((pl.NPC_PAD, D), dtype=np.float32)
        for (g, cidx, cb, nc_, io, qn) in pl.gathers:
            n16 = nc_ * P // 16
            pb = qn * 32
            flat = mm["idx16"][pb : pb + 16, io : io + n16].T.reshape(-1)
            ids = flat.reshape(nc_, P).astype(np.int64) + cidx * RSZ
            c0 = pl.col_of_grp[g] + cb
            for j in range(nc_):
                gcol = c0 + j
                t = int(np.nonzero(mm["ind"][:, gcol])[0][0])
                rows = ids[j]
                xs = tabx[rows].astype(np.float32)
                p2s = p2[rows].astype(np.float32)
                lanes = np.arange(P) + t * P
                arg = np.float16(p2s + p1b[lanes]).astype(np.float32)
                ee = np.float16(
                    np.tanh(arg) * mm["npr"][:, gcol].astype(np.float32)
                )
                z[lanes] += np.float16(
                    xs * ee.astype(np.float32)[:, None]
                ).astype(np.float32)
        outs.append(z)
    return outs


# ---------------------------------------------------------------------------
# device program
# ---------------------------------------------------------------------------

def _dma_gather(nc, mybir, out_ap, in_ap, idxs_ap, num_idxs, queue_num):
    gp = nc.gpsimd
    from concourse.bass import exact_div

    stride_bytes_256 = exact_div(FS * mybir.dt.size(in_ap.dtype), 256)
    _in_ap = gp.lower_ap_dma(in_ap, for_custom_bir_dma=True)
    _idxs_ap = gp.lower_ap(idxs_ap)
    _out_ap = gp.lower_ap(out_ap)
    return gp.add_instruction(
        mybir.InstDMAGatherAnt(
            name=gp.bass.get_next_instruction_name(),
            ins=[*_in_ap, _idxs_ap, gp.lower_val_access(gp.to_reg(num_idxs))],
            outs=[_out_ap],
            transpose=False,
            num_idxs=num_idxs,
            elem_size=ELEM,
            stride_bytes_256=stride_bytes_256,
            gen_mode=0,
            single_packet=True,
            queue_num=queue_num,
            sbuf_tokens_per_rank=0,
            sbuf_free_dim_per_rank=0,
            sbuf_free_dim_pad_per_rank=0,
            sbuf_byte_offset=0,
        )
    )


def _build_nc(pl):
    import concourse.bacc as bacc
    import concourse.mybir as mybir
    import concourse.tile as tile
    from concourse.masks import make_identity

    f32 = mybir.dt.float32
    f16 = mybir.dt.float16
    i16 = mybir.dt.int16
    AF = mybir.ActivationFunctionType
    OP = mybir.AluOpType
    AX = mybir.AxisListType

    TILES, NPC_PAD = pl.TILES, pl.NPC_PAD

    nc = bacc.Bacc(
        "TRN2",
        target_bir_lowering=False,
        debug=False,
        num_devices=NCORES,
        num_swdge_queues=4,
    )
    xp_d = nc.dram_tensor("xp", [NPAD, D], f32, kind="ExternalInput")
    w16_d = nc.dram_tensor("wrep16", [P, D], f16, kind="ExternalInput")
    w32_d = nc.dram_tensor("wrep32", [P, D], f32, kind="ExternalInput")
    b128_d = nc.dram_tensor("b128", [P, 1], f32, kind="ExternalInput")
    xown_d = nc.dram_tensor("xown", [NPC_PAD, D], f32, kind="ExternalInput")
    idx_d = nc.dram_tensor("idx16", [P, pl.IDX16], i16, kind="ExternalInput")
    npr_d = nc.dram_tensor("npr", [P, pl.CTOT], f16, kind="ExternalInput")
    ind_d = nc.dram_tensor("ind", [P, pl.CTOT], f16, kind="ExternalInput")
    z_d = nc.dram_tensor("z", [NPC_PAD, D], f32, kind="ExternalOutput")
    tab_d = nc.dram_tensor("tab", [NPAD, FS], f16)

    with tile.TileContext(nc) as tc, nc.allow_low_precision("fp16 messages"):
        with tc.tile_pool(name="consts", bufs=1) as cpool:
            w16_sb = cpool.tile([P, D], f16)
            nc.sync.dma_start(out=w16_sb[:], in_=w16_d[:, :])
            w32_sb = cpool.tile([P, D], f32)
            nc.sync.dma_start(out=w32_sb[:], in_=w32_d[:, :])
            b128_sb = cpool.tile([P, 1], f32)
            nc.sync.dma_start(out=b128_sb[:], in_=b128_d[:, :])
            idx_sb = cpool.tile([P, pl.IDX16], i16)
            nc.sync.dma_start(out=idx_sb[:], in_=idx_d[:, :])
            npr_sb = cpool.tile([P, pl.CTOT], f16)
            nc.sync.dma_start(out=npr_sb[:], in_=npr_d[:, :])
            ind_sb = cpool.tile([P, pl.CTOT], f16)
            nc.sync.dma_start(out=ind_sb[:], in_=ind_d[:, :])
            p1bT = cpool.tile([P, P], f16)
            p1b_sb = cpool.tile([P, TILES], f32)

            # ---- phase 1 --------------------------------------------------
            with tc.tile_pool(name="rowb", bufs=1) as rpool:
                rowbuf = rpool.tile([P, IPC * ELEM], f16)
                rbv = rowbuf[:].rearrange("p (i f) -> p i f", f=ELEM)
                BC = 56
                xpv = xp_d[:, :].rearrange("(p i) f -> p i f", p=P)
                with tc.tile_pool(name="ph1a", bufs=2) as papool:
                    for c0 in range(0, IPC, BC):
                        cn = min(BC, IPC - c0)
                        nc.gpsimd.dma_start(
                            out=rbv[:, c0 : c0 + cn, 0:D],
                            in_=xpv[:, c0 : c0 + cn, :],
                        )
                        tmp = papool.tile([P, BC * D], f16, tag="tmp")
                        tv = tmp[:].rearrange("p (i f) -> p i f", f=D)
                        nc.vector.tensor_tensor(
                            out=tv[:, 0:cn, :],
                            in0=rbv[:, c0 : c0 + cn, 0:D],
                            in1=w16_sb[:]
                            .rearrange("p (o f) -> p o f", o=1)
                            .to_broadcast([P, cn, D]),
                            op=OP.mult,
                        )
                        nc.vector.tensor_reduce(
                            out=rbv[:, c0 : c0 + cn, D],
                            in_=tv[:, 0:cn, :],
                            axis=AX.X,
                            op=OP.add,
                        )
                tabv = tab_d[:, 0:ELEM].rearrange("(p i) f -> p i f", p=P)
                for c in range(NCOLOR):
                    nc.sync.dma_start(
                        out=tabv[c * CPP : (c + 1) * CPP, :, :],
                        in_=rbv[c * CPP : (c + 1) * CPP, :, :],
                    )

            # ---- phase 1b: p1b + PE transpose -----------------------------
            with tc.tile_pool(name="ph1", bufs=1) as p1pool:
                xo = p1pool.tile([P, TILES * D], f32, tag="xo")
                xov = xo[:].rearrange("p (t f) -> p t f", f=D)
                nc.sync.dma_start(
                    out=xov[:, :, :],
                    in_=xown_d[:, :].rearrange("(t p) f -> p t f", t=TILES),
                )
                tmp2 = p1pool.tile([P, TILES * D], f32, tag="tmp2")
                t2v = tmp2[:].rearrange("p (t f) -> p t f", f=D)
                nc.vector.tensor_tensor(
                    out=t2v[:, :, :],
                    in0=xov[:, :, :],
                    in1=w32_sb[:]
                    .rearrange("p (o f) -> p o f", o=1)
                    .to_broadcast([P, TILES, D]),
                    op=OP.mult,
                )
                red = p1pool.tile([P, TILES], f32, tag="red")
                nc.vector.tensor_reduce(
                    out=red[:], in_=t2v[:, :, :], axis=AX.X, op=OP.add
                )
                nc.vector.tensor_scalar(
                    out=p1b_sb[:], in0=red[:], scalar1=b128_sb[:, 0:1],
                    scalar2=None, op0=OP.add,
                )
                with tc.tile_pool(name="ps_t", bufs=1, space="PSUM") as ps_t:
                    ident = p1pool.tile([P, P], f32, tag="ident")
                    make_identity(nc, ident[:])
                    p1bT_ps = ps_t.tile([P, P], f32, tag="p1bt")
                    nc.tensor.transpose(
                        out=p1bT_ps[0:TILES, 0:P],
                        in_=p1b_sb[:, 0:TILES],
                        identity=ident[:],
                    )
                    nc.vector.tensor_copy(
                        out=p1bT[0:TILES, :], in_=p1bT_ps[0:TILES, 0:P]
                    )

            # ---- phase 2 --------------------------------------------------
            gidx = 0
            with (
                tc.tile_pool(name="ga", bufs=2) as gapool,
                tc.tile_pool(name="mm", bufs=1) as mpool,
                tc.tile_pool(name="sc", bufs=2) as spool,
                tc.tile_pool(name="ps", bufs=2, space="PSUM") as pspool,
            ):
                for g, (a, b) in enumerate(pl.groups):
                    G = b - a
                    C = pl.grp_cols[g]
                    cb0 = int(pl.col_of_grp[g])
                    ga = gapool.tile([P, C * ELEM], f16, tag="ga")
                    gav = ga[:].rearrange("p (c f) -> p c f", f=ELEM)
                    while gidx < len(pl.gathers) and pl.gathers[gidx][0] == g:
                        (_, cidx, cb, nc_, io, qn) = pl.gathers[gidx]
                        _dma_gather(
                            nc, mybir,
                            out_ap=gav[:, cb : cb + nc_, :],
                            in_ap=tab_d[cidx * RSZ : (cidx + 1) * RSZ, 0:ELEM],
                            idxs_ap=idx_sb[:, io : io + nc_ * P // 16],
                            num_idxs=nc_ * P,
                            queue_num=qn,
                        )
                        gidx += 1

                    pcols = pspool.tile([P, C], f32, tag="pcols")
                    nc.tensor.matmul(
                        out=pcols[:],
                        lhsT=p1bT[0:TILES, :],
                        rhs=ind_sb[0:TILES, cb0 : cb0 + C],
                    )
                    arg = spool.tile([P, C], f16, tag="arg")
                    nc.vector.tensor_tensor(
                        out=arg[:], in0=gav[:, :, D], in1=pcols[:], op=OP.add
                    )
                    tt = spool.tile([P, C], f16, tag="tt")
                    nc.scalar.activation(out=tt[:], in_=arg[:], func=AF.Tanh)
                    ee = spool.tile([P, C], f16, tag="ee")
                    nc.vector.tensor_tensor(
                        out=ee[:], in0=tt[:], in1=npr_sb[:, cb0 : cb0 + C],
                        op=OP.mult,
                    )
                    m = mpool.tile([P, C * D], f16, tag="m")
                    mv = m[:].rearrange("p (c f) -> p c f", f=D)
                    nc.vector.tensor_tensor(
                        out=mv[:, :, :],
                        in0=gav[:, :, 0:D],
                        in1=ee[:].rearrange("p (c o) -> p c o", o=1)
                        .to_broadcast([P, C, D]),
                        op=OP.mult,
                    )
                    zpart = spool.tile([P, NCOLOR * G * D], f16, tag="zp")
                    zpv = zpart[:].rearrange(
                        "p (r t f) -> p r t f", r=NCOLOR, f=D
                    )
                    for cidx in range(NCOLOR):
                        # equal-K runs of tiles in natural order
                        tl = pl.colmap[g][cidx]
                        i = 0
                        while i < len(tl):
                            t_i, cb_i = tl[i]
                            K = int(pl.Ktc[t_i, cidx])
                            j = i + 1
                            while j < len(tl) and int(
                                pl.Ktc[tl[j][0], cidx]
                            ) == K:
                                j += 1
                            RL = j - i
                            mseg = m[
                                :, cb_i * D : (cb_i + RL * K) * D
                            ].rearrange("p (t k f) -> p t k f", t=RL, f=D)
                            k = K
                            while k > 1:
                                h2 = 1 << (k.bit_length() - 1)
                                if h2 == k:
                                    h2 = k // 2
                                r = k - h2
                                nc.vector.tensor_tensor(
                                    out=mseg[:, :, 0:r, :],
                                    in0=mseg[:, :, 0:r, :],
                                    in1=mseg[:, :, h2:k, :],
                                    op=OP.add,
                                )
                                k = h2
                            nc.vector.tensor_copy(
                                out=zpv[:, cidx, t_i - a : t_i - a + RL, :],
                                in_=mseg[:, :, 0, :],
                            )
                            i = j
                    nc.vector.tensor_tensor(
                        out=zpv[:, 0, :, :], in0=zpv[:, 0, :, :],
                        in1=zpv[:, 1, :, :], op=OP.add,
                    )
                    nc.vector.tensor_tensor(
                        out=zpv[:, 2, :, :], in0=zpv[:, 2, :, :],
                        in1=zpv[:, 3, :, :], op=OP.add,
                    )
                    zt = spool.tile([P, G * D], f32, tag="zt")
                    nc.vector.tensor_tensor(
                        out=zt[:].rearrange("p (t f) -> p t f", f=D),
                        in0=zpv[:, 0, :, :], in1=zpv[:, 2, :, :], op=OP.add,
                    )
                    nc.sync.dma_start(
                        out=z_d[a * P : b * P, :].rearrange(
                            "(t p) f -> p t f", t=G
                        ),
                        in_=zt[:].rearrange("p (t f) -> p t f", f=D),
                    )
    nc.compile()
    return nc


_BUILD_CACHE = {}


def _assemble(pl, outs):
    z = np.zeros((pl.N, D), dtype=np.float32)
    for c in range(NCORES):
        nodes = pl.core_nodes[c]
        real = nodes >= 0
        z[nodes[real]] = outs[c][real]
    return z


def kernel(x, gate_w, gate_b, src, dst):
    global LAST_RESULTS
    pl, in_maps = _prep(x, gate_w, gate_b, src, dst)
    if os.environ.get("FAGCN_EMU"):
        return _assemble(pl, emulate(pl, in_maps))
    from concourse.bass_utils import run_bass_kernel_spmd

    key = (pl.N, pl.E, tuple(pl.grp_cols))
    nc = _BUILD_CACHE.get(key)
    if nc is None:
        nc = _build_nc(pl)
        _BUILD_CACHE[key] = nc
    res = run_bass_kernel_spmd(
        nc,
        in_maps,
        core_ids=list(range(NCORES)),
        trace=bool(int(os.environ.get("FAGCN_TRACE", "0"))),
    )
    LAST_RESULTS = res
    outs = [r["z"] for r in res.results]
    return _assemble(pl, outs)
